# revision 27
# baseline (speedup 1.0000x reference)
"""BiLSTM (2-layer, H=50, D=207, T=30, B=16384) -> FC(2) Trainium2 kernel.

Data-parallel over 8 NeuronCores (2048 batch rows each). Host pre-packs
x into feature-major fp16 layout [T, 208, B] (ones row folds biases into
the input projection) and reorders weights into PSUM-bank gate layout.

v2 layout (per core, 4 sweeps of 512 batch):
  - fp16 everywhere on SBUF (PSUM accumulates f32); DVE tensor ops run in
    2x packed mode on 16-bit operands.
  - g-gate weight columns pre-scaled x2 so ONE 4-bank sigmoid ACTIVATE
    yields i,f,o and s=sigmoid(2g)=(tanh(g)+1)/2; the cell fixes up with
    fused scalar_tensor_tensor ops: t=(s-0.5)*i, c=f*c, c=2t+c.
  - x DMAs: one [128/80, 2048] transfer per (step, kind) covering all 4
    sweeps; fwd loads issue on the sync queue, bwd on gpsimd.
  - h1 history DMAs split across sync (fwd half) / gpsimd (bwd half).
  - Matmul order interleaves quadrant pairs (col-group concurrency) and
    puts same-weight matmuls for the two in-flight sweeps back to back.
"""

import numpy as np

import concourse.bass as bass
import concourse.tile as tile
from concourse import bacc, mybir
from concourse.bass_utils import run_bass_kernel_spmd

F16 = mybir.dt.float16
F32 = mybir.dt.float32
AF = mybir.ActivationFunctionType
OP = mybir.AluOpType

H = 50
DIN = 207
DK = 208          # D + ones row
T = 30
B = 16384
NCORES = 8
BC = B // NCORES  # 2048
NSW = 4           # sweeps per core
BS = 512          # sweep batch size
Q = 64            # quadrant offset for second lane (dir-b / sweep s2)
KH = 114          # K rows for rec matmuls / h span (0:50 real, 64:114 real)
KH1 = 115         # K rows for L1 proj (incl. ones row at 114)

# PyTorch LSTM gate order in weight rows: i, f, g, o — bank order matches.
GATE_SLICES = [slice(0, 50), slice(50, 100), slice(100, 150), slice(150, 200)]
# g-bank pre-activations are scaled x2 so sigmoid(2g) = (tanh(g)+1)/2.
GATE_SCALE = [1.0, 1.0, 2.0, 1.0]


def _pack_weights(inp):
    f32 = np.float32
    # ---- L0 projection (+bias via ones row), [208, 4*128] ----
    w0 = np.zeros((DK, 512), f32)
    wf = np.concatenate([inp["wih0f"], inp["b0f"][:, None]], axis=1)  # [200,208]
    wb = np.concatenate([inp["wih0b"], inp["b0b"][:, None]], axis=1)
    for c, gs in enumerate(GATE_SLICES):
        w0[:, c * 128 + 0:c * 128 + 50] = GATE_SCALE[c] * wf[gs].T
        w0[:, c * 128 + Q:c * 128 + Q + 50] = GATE_SCALE[c] * wb[gs].T
    # ---- L0 recurrent, block-diag [114, 4*128] ----
    r0 = np.zeros((KH, 512), f32)
    for c, gs in enumerate(GATE_SLICES):
        r0[0:50, c * 128 + 0:c * 128 + 50] = GATE_SCALE[c] * inp["whh0f"][gs].T
        r0[Q:Q + 50, c * 128 + Q:c * 128 + Q + 50] = GATE_SCALE[c] * inp["whh0b"][gs].T
    # ---- L1 fwd projection [115, 4*64] (rows: h1f 0:50, h1b 64:114, bias 114)
    def l1_proj(wih, b):
        w = np.zeros((KH1, 256), f32)
        for c, gs in enumerate(GATE_SLICES):
            w[0:50, c * 64:c * 64 + 50] = GATE_SCALE[c] * wih[gs, 0:50].T
            w[Q:Q + 50, c * 64:c * 64 + 50] = GATE_SCALE[c] * wih[gs, 50:100].T
            w[KH, c * 64:c * 64 + 50] = GATE_SCALE[c] * b[gs]
        return w
    w1 = l1_proj(inp["wih1f"], inp["b1f"])
    w1b = l1_proj(inp["wih1b"], inp["b1b"])
    # ---- L1 recurrent, block-diag per sweep pair [114, 4*128] ----
    r1 = np.zeros((KH, 512), f32)
    for c, gs in enumerate(GATE_SLICES):
        r1[0:50, c * 128 + 0:c * 128 + 50] = GATE_SCALE[c] * inp["whh1f"][gs].T
        r1[Q:Q + 50, c * 128 + Q:c * 128 + Q + 50] = GATE_SCALE[c] * inp["whh1f"][gs].T
    # ---- FC ----
    # FC weights pre-staged at both quadrants host-side: a [50,2] DMA has
    # 4-byte lines and cost ~13us on hardware; one [128,2] DMA is cheap.
    wffq = np.zeros((128, 2), f32)
    wfbq = np.zeros((128, 2), f32)
    wffq[0:50] = wffq[Q:Q + 50] = inp["fc_w"][:, 0:50].T
    wfbq[0:50] = wfbq[Q:Q + 50] = inp["fc_w"][:, 50:100].T
    f16 = np.float16
    return {
        "w0hi": w0[0:128].astype(f16), "w0lo": w0[128:DK].astype(f16),
        "r0": r0.astype(f16), "w1": w1.astype(f16), "w1b": w1b.astype(f16),
        "r1": r1.astype(f16), "wffq": wffq.astype(f16), "wfbq": wfbq.astype(f16),
        # h1 scratch-row init blocks, DMA'd instead of memset (engines are
        # slow at big memsets; DMA rings are idle at startup).
        "h1z": np.zeros((32, T, BS), f16),
        "h1o": np.ones((32, T, BS), f16),
    }


def _pack_x(x):
    # x [B, T, 207] f32 -> [T, 208, B] fp16 with ones row at d=207
    xt = np.empty((T, DK, B), dtype=np.float16)
    xt[:, 0:DIN, :] = x.transpose(1, 2, 0).astype(np.float16)
    xt[:, DIN, :] = np.float16(1.0)
    return xt


def _elide_redundant_ldweights(nc):
    """Remove InstLdweights whose weights are already resident in the same
    PE col-range (bass emits one LDWEIGHTS per matmul unconditionally).

    Back-to-back same-weight matmuls then pipeline fill-behind-drain
    instead of paying the full drain + reload between them. Waits/updates
    on a removed LDW migrate to the next PE instruction (its matmul)."""
    removed = skipped = 0
    for blk in nc.m.functions[0].blocks:
        insts = blk.instructions
        state = {}  # col_start -> (width, signature)
        to_remove = []
        for idx, inst in enumerate(insts):
            if not isinstance(inst, mybir.InstLdweights):
                continue
            tp = getattr(inst, "tile_position", None) or (0, 0)
            tsz = getattr(inst, "tile_size", None) or (128, 128)
            col0, width = tp[1], tsz[1]
            sig = (str(inst.ins[0]), str(getattr(inst, "perf_mode", None)),
                   str(getattr(inst, "is_transpose", None)), width)
            prev = state.get(col0)
            if prev == sig:
                # find next PE instruction (the matmul this LDW feeds)
                si = inst.sync_info
                nxt = None
                for j in range(idx + 1, len(insts)):
                    if insts[j].engine == mybir.EngineType.PE:
                        nxt = insts[j]
                        break
                if nxt is None:
                    skipped += 1
                    continue
                nsi = nxt.sync_info
                lw = list(si.on_wait) if si else []
                lu = list(si.on_update) if si else []
                nw = list(nsi.on_wait) if nsi else []
                if lw and nw:
                    skipped += 1  # can't merge two waits; keep the LDW
                    continue
                if lw or lu:
                    nu = list(nsi.on_update) if nsi else []
                    nxt.sync_info = mybir.SyncInfo(
                        on_wait=nw + lw, on_update=nu + lu)
                to_remove.append(idx)
            else:
                # invalidate anything overlapping this col range, then record
                for c0 in list(state):
                    w0 = state[c0][3] if isinstance(state[c0], tuple) else 128
                    if c0 < col0 + width and col0 < c0 + w0:
                        del state[c0]
                state[col0] = sig
        for idx in reversed(to_remove):
            del insts[idx]
        removed += len(to_remove)
    print(f"ldweights elision: removed {removed}, kept-for-waits {skipped}")


def _build_nc():
    nc = bacc.Bacc("TRN2", target_bir_lowering=False, debug=False)
    ap = {}
    ap["xT"] = nc.dram_tensor("xT", [T, DK, BC], F16, kind="ExternalInput").ap()
    for name, shp in [("w0hi", [128, 512]), ("w0lo", [DK - 128, 512]),
                      ("r0", [KH, 512]), ("w1", [KH1, 256]), ("w1b", [KH1, 256]),
                      ("r1", [KH, 512]), ("wffq", [128, 2]), ("wfbq", [128, 2]),
                      ("h1z", [32, T, BS]), ("h1o", [32, T, BS])]:
        ap[name] = nc.dram_tensor(name, shp, F16, kind="ExternalInput").ap()
    out_ap = nc.dram_tensor("out", [2, BC], F32, kind="ExternalOutput").ap()

    with tile.TileContext(nc) as tc:
        with (
            tc.tile_pool(name="wts", bufs=1) as wp,
            tc.tile_pool(name="xin", bufs=2) as xp,
            tc.tile_pool(name="h1p", bufs=1) as h1p,
            tc.tile_pool(name="st", bufs=1) as sp,
            tc.tile_pool(name="gt", bufs=1) as gp,
            tc.tile_pool(name="outp", bufs=2) as op_,
        ):
            # ---- load weights to SBUF ----
            # Phase-A weights on sync (needed immediately, ahead of x loads);
            # layer-1 weights on gpsimd; FC weights on the idle scalar queue
            # (a slow 4-byte-line DMA there blocked startup for ~19us).
            w = {}
            for name, shp, eng in [
                ("w0hi", [128, 512], nc.sync), ("w0lo", [DK - 128, 512], nc.sync),
                ("r0", [KH, 512], nc.sync), ("w1", [KH1, 256], nc.gpsimd),
                ("w1b", [KH1, 256], nc.gpsimd), ("r1", [KH, 512], nc.gpsimd),
                ("wffq", [128, 2], nc.scalar), ("wfbq", [128, 2], nc.scalar),
            ]:
                t = wp.tile(shp, F16, tag=name, name=name)
                eng.dma_start(t[:], ap[name][:])
                w[name] = t

            # ---- h1 history per sweep [115, T, 512]; row 114 = ones ----
            h1 = []
            for s in range(NSW):
                t = h1p.tile([128, T, BS], F16, tag=f"h1_{s}", name=f"h1_{s}")
                # ones row lives at 114; init needs a 32-aligned partition
                # start, so fill 96:128 — Phase A's h1 DMA re-writes 96:114
                # before layer 1 reads, and 115:128 is outside the K span.
                # DMA'd from host constants: engine memsets of this size cost
                # ~13us each and serialize startup; DMA rings are idle here.
                eng = nc.scalar if s < 2 else nc.gpsimd
                eng.dma_start(t[96:128, :, :], ap["h1o"][:])
                # rows 50:64 are never DMA'd; zero them so the 0-weight
                # K rows in layer-1 matmuls read 0 (not NaN). 32:50 gets
                # overwritten by the fwd-lane h1 DMA each step.
                eng.dma_start(t[32:64, :, :], ap["h1z"][:])
                h1.append(t)

            # ---- states (all fp16 for 2x DVE modes) ----
            hS = [sp.tile([128, BS], F16, tag=f"hs{s}", name=f"hs{s}") for s in range(NSW)]
            # c / tanh(c) live in per-pair tiles so one ACTIVATE covers both
            # in-flight sweeps of a pair.
            cP = [sp.tile([128, 2, BS], F16, tag=f"cp{r}", name=f"cp{r}") for r in range(2)]
            tP = [sp.tile([128, 2, BS], F16, tag=f"tp{r}", name=f"tp{r}") for r in range(2)]
            hB = [sp.tile([128, BS], F16, tag=f"hb{p}", name=f"hb{p}") for p in range(2)]
            hC = [sp.tile([128, BS], F16, tag=f"hc{p}", name=f"hc{p}") for p in range(2)]
            cB = [sp.tile([128, BS], F16, tag=f"cb{p}", name=f"cb{p}") for p in range(2)]
            tB = [sp.tile([128, BS], F16, tag=f"tb{p}", name=f"tb{p}") for p in range(2)]
            cC = [sp.tile([128, BS], F16, tag=f"cc{p}", name=f"cc{p}") for p in range(2)]
            tC = [sp.tile([128, BS], F16, tag=f"tc{p}", name=f"tc{p}") for p in range(2)]

            def cell_cupd(G, c_t, k):
                """c'-update on a HALF-SCALED cell state c' = c/2:
                t = (s-0.5)*i == i*tanh(g)/2, c' = f*c' + t. The x2 rides
                tanh's free scale later. g-weights are pre-scaled x2 so the
                g-bank sigmoid s gives tanh(g) = 2s-1. Plain tensor_tensor
                runs at 2x on fp16; scalar_tensor_tensor only has a 1x uop,
                so keep just one of those."""
                i_, f_, g_ = (G[0:KH, j, :] for j in range(3))
                c_ = c_t[0:KH, :]
                if k == 0:
                    nc.vector.scalar_tensor_tensor(c_, g_, 0.5, i_,
                                                   OP.subtract, OP.mult)
                else:
                    nc.vector.tensor_mul(c_, f_, c_)
                    nc.vector.scalar_tensor_tensor(g_, g_, 0.5, i_,
                                                   OP.subtract, OP.mult)
                    nc.vector.tensor_add(c_, c_, g_)

            def cell(Pif, Pgo, G, c_t, k):
                """Split-PSUM cell front: two 2-bank sigmoids (i,f) / (2g,o)
                so f*c can start as soon as the first one lands. tanh + h
                are emitted by the caller (batched per pair in phase A)."""
                nc.scalar.activation(G[0:KH, 0:2, :], Pif[0:KH, :, :], AF.Sigmoid)
                nc.scalar.activation(G[0:KH, 2:4, :], Pgo[0:KH, :, :], AF.Sigmoid)
                cell_cupd(G, c_t, k)

            def cell42(P4, G, c_t, th_t, h_out, k):
                """Phase B/C cell on a single 4-bank tile, sigmoid split in
                two so the c-chain starts after the first half (the layer-1
                scan is latency-bound, not ACT-throughput-bound)."""
                nc.scalar.activation(G[0:KH, 0:2, :], P4[0:KH, 0:2, :], AF.Sigmoid)
                nc.scalar.activation(G[0:KH, 2:4, :], P4[0:KH, 2:4, :], AF.Sigmoid)
                cell_cupd(G, c_t, k)
                nc.scalar.activation(th_t[0:KH, :], c_t[0:KH, :], AF.Tanh,
                                     scale=2.0)
                nc.vector.tensor_mul(h_out[0:KH, :], G[0:KH, 3, :],
                                     th_t[0:KH, :])

            with (
                tc.tile_pool(name="psA", bufs=2, space="PSUM") as ppA,
                tc.tile_pool(name="psB", bufs=2, space="PSUM") as ppB,
            ):
                def gate_tiles():
                    """Two half-tiles per sweep: banks (i,f) and (g,o).
                    Each recycles as soon as its own sigmoid drains it."""
                    pif = ppA.tile([128, 2, BS], F32, tag="pif", name="pif")
                    pgo = ppB.tile([128, 2, BS], F32, tag="pgo", name="pgo")
                    return pif, pgo

                def bank(pair, c, rows):
                    pif, pgo = pair
                    return (pif if c < 2 else pgo)[rows, c % 2, :]

                # =================== Phase A: layer 0, lockstep ===========
                for k in range(T):
                    # one x DMA per (step, kind) covering all 4 sweeps;
                    # fwd loads on sync queue, bwd loads on gpsimd queue.
                    xfh = xp.tile([128, BC], F16, tag="xfh", name="xfh")
                    nc.sync.dma_start(xfh[:], ap["xT"][k, 0:128, :])
                    xfl = xp.tile([DK - 128, BC], F16, tag="xfl", name="xfl")
                    nc.sync.dma_start(xfl[:], ap["xT"][k, 128:DK, :])
                    xbh = xp.tile([128, BC], F16, tag="xbh", name="xbh")
                    nc.gpsimd.dma_start(xbh[:], ap["xT"][T - 1 - k, 0:128, :])
                    xbl = xp.tile([DK - 128, BC], F16, tag="xbl", name="xbl")
                    nc.gpsimd.dma_start(xbl[:], ap["xT"][T - 1 - k, 128:DK, :])

                    for pr in range(2):
                        sa, sb = 2 * pr, 2 * pr + 1
                        Ps = [gate_tiles() for _ in range(2)]
                        # Rotated interleave: adjacent MMs hit disjoint
                        # col-groups (concurrent); same-weight MMs for the
                        # two in-flight sweeps are back to back.
                        for wt_, xfx, xbx, st in ((w["w0hi"], xfh, xbh, True),
                                                  (w["w0lo"], xfl, xbl, False)):
                            for c in range(4):
                                cb = (c + 2) % 4
                                spA = k == 0 and not st and c >= 2
                                spB = k == 0 and not st and cb < 2
                                for sj, s in enumerate((sa, sb)):
                                    sl = bass.ts(s, BS)
                                    nc.tensor.matmul(
                                        bank(Ps[sj], c, slice(0, Q)),
                                        wt_[:, bass.ds(c * 128, Q)], xfx[:, sl],
                                        start=st, stop=spA)
                                    nc.tensor.matmul(
                                        bank(Ps[sj], cb, slice(Q, 128)),
                                        wt_[:, bass.ds(cb * 128 + Q, Q)],
                                        xbx[:, sl],
                                        start=st, stop=spB)
                        if k > 0:
                            for c in range(4):
                                for sj, s in enumerate((sa, sb)):
                                    nc.tensor.matmul(
                                        bank(Ps[sj], c, slice(0, 128)),
                                        w["r0"][:, bass.ts(c, 128)],
                                        hS[s][0:KH, :],
                                        start=False, stop=True)

                        Gs = []
                        for sj, s in enumerate((sa, sb)):
                            G = gp.tile([128, 4, BS], F16, tag=f"g{s}", name=f"g{s}")
                            Gs.append(G)
                            cell(Ps[sj][0], Ps[sj][1], G, cP[pr][:, sj, :], k)
                        # tanh over both sweeps of the pair in one ACTIVATE
                        # (x2 undoes the half-scaled c' state).
                        nc.scalar.activation(tP[pr][0:KH, :, :],
                                             cP[pr][0:KH, :, :], AF.Tanh,
                                             scale=2.0)
                        for sj, s in enumerate((sa, sb)):
                            nc.vector.tensor_mul(hS[s][0:KH, :],
                                                 Gs[sj][0:KH, 3, :],
                                                 tP[pr][0:KH, sj, :])
                            # h1 history is time-aligned: bwd lane at step k
                            # holds h_b for time T-1-k.
                            nc.sync.dma_start(h1[s][0:50, k, :], hS[s][0:50, :])
                            nc.gpsimd.dma_start(h1[s][Q:Q + 50, T - 1 - k, :],
                                                hS[s][Q:Q + 50, :])

            # =================== Phase B: layer 1 forward =============
            with tc.tile_pool(name="ps4", bufs=2, space="PSUM") as pp4:
                for k in range(T):
                    Ps = [pp4.tile([128, 4, BS], F32, tag="p4", name="p4")
                          for _ in range(2)]
                    for c in range(4):
                        cb = (c + 2) % 4
                        for p in range(2):
                            s1, s2 = 2 * p, 2 * p + 1
                            nc.tensor.matmul(
                                Ps[p][0:Q, c, :],
                                w["w1"][:, bass.ts(c, 64)],
                                h1[s1][0:KH1, k, :], start=True,
                                stop=(k == 0 and c >= 2))
                            nc.tensor.matmul(
                                Ps[p][Q:128, cb, :],
                                w["w1"][:, bass.ts(cb, 64)],
                                h1[s2][0:KH1, k, :], start=True,
                                stop=(k == 0 and cb < 2))
                    if k > 0:
                        for c in range(4):
                            for p in range(2):
                                nc.tensor.matmul(
                                    Ps[p][:, c, :],
                                    w["r1"][:, bass.ts(c, 128)], hB[p][0:KH, :],
                                    start=False, stop=True)
                    for p in range(2):
                        G = gp.tile([128, 4, BS], F16, tag=f"g{p}", name=f"g{p}")
                        cell42(Ps[p], G, cB[p], tB[p], hB[p], k)

                # =================== Phase C: layer 1 backward (t=29) =====
                for p in range(2):
                    s1, s2 = 2 * p, 2 * p + 1
                    P = pp4.tile([128, 4, BS], F32, tag="p4", name="p4")
                    for c in range(4):
                        cb = (c + 2) % 4
                        nc.tensor.matmul(
                            P[0:Q, c, :], w["w1b"][:, bass.ts(c, 64)],
                            h1[s1][0:KH1, T - 1, :], start=True, stop=c >= 2)
                        nc.tensor.matmul(
                            P[Q:128, cb, :], w["w1b"][:, bass.ts(cb, 64)],
                            h1[s2][0:KH1, T - 1, :], start=True, stop=cb < 2)
                    G = gp.tile([128, 4, BS], F16, tag=f"g{p + 2}", name=f"g{p + 2}")
                    cell42(P, G, cC[p], tC[p], hC[p], 0)

            # =================== FC ====================================
            with tc.tile_pool(name="fcp", bufs=4, space="PSUM") as fcp:
                for s in range(NSW):
                    p, qo = s // 2, (s % 2) * Q
                    F = fcp.tile([2, BS], F32, tag="fc", name="fc")
                    nc.tensor.matmul(F[:], w["wffq"][qo:qo + 50, :],
                                     hB[p][qo:qo + 50, :], start=True, stop=False)
                    nc.tensor.matmul(F[:], w["wfbq"][qo:qo + 50, :],
                                     hC[p][qo:qo + 50, :], start=False, stop=True)
                    ot = op_.tile([2, BS], F32, tag="o", name="ot")
                    nc.vector.tensor_copy(ot[:], F[:])
                    nc.sync.dma_start(out_ap[:, bass.ts(s, BS)], ot[:])

    nc.compile()
    _elide_redundant_ldweights(nc)
    return nc


_NC_CACHE = None


def kernel(**inputs) -> np.ndarray:
    global _NC_CACHE
    if _NC_CACHE is None:
        _NC_CACHE = _build_nc()
    nc = _NC_CACHE
    wts = _pack_weights(inputs)
    xt = _pack_x(np.asarray(inputs["x"], dtype=np.float32))
    in_maps = []
    for c in range(NCORES):
        m = dict(wts)
        m["xT"] = np.ascontiguousarray(xt[:, :, c * BC:(c + 1) * BC])
        in_maps.append(m)
    res = run_bass_kernel_spmd(nc, in_maps, list(range(NCORES)))
    outs = [res.results[c]["out"] for c in range(NCORES)]  # [2, BC] each
    full = np.concatenate(outs, axis=1).T  # [B, 2]
    return (full + inputs["fc_b"][None, :]).astype(np.float32)


# revision 28
# speedup vs baseline: 1.0021x; 1.0021x over previous
"""BiLSTM (2-layer, H=50, D=207, T=30, B=16384) -> FC(2) Trainium2 kernel.

Data-parallel over 8 NeuronCores (2048 batch rows each). Host pre-packs
x into feature-major fp16 layout [T, 208, B] (ones row folds biases into
the input projection) and reorders weights into PSUM-bank gate layout.

v2 layout (per core, 4 sweeps of 512 batch):
  - fp16 everywhere on SBUF (PSUM accumulates f32); DVE tensor ops run in
    2x packed mode on 16-bit operands.
  - g-gate weight columns pre-scaled x2 so ONE 4-bank sigmoid ACTIVATE
    yields i,f,o and s=sigmoid(2g)=(tanh(g)+1)/2; the cell fixes up with
    fused scalar_tensor_tensor ops: t=(s-0.5)*i, c=f*c, c=2t+c.
  - x DMAs: one [128/80, 2048] transfer per (step, kind) covering all 4
    sweeps; fwd loads issue on the sync queue, bwd on gpsimd.
  - h1 history DMAs split across sync (fwd half) / gpsimd (bwd half).
  - Matmul order interleaves quadrant pairs (col-group concurrency) and
    puts same-weight matmuls for the two in-flight sweeps back to back.
"""

import numpy as np

import concourse.bass as bass
import concourse.tile as tile
from concourse import bacc, mybir
from concourse.bass_utils import run_bass_kernel_spmd

F16 = mybir.dt.float16
F32 = mybir.dt.float32
AF = mybir.ActivationFunctionType
OP = mybir.AluOpType

H = 50
DIN = 207
DK = 208          # D + ones row
T = 30
B = 16384
NCORES = 8
BC = B // NCORES  # 2048
NSW = 4           # sweeps per core
BS = 512          # sweep batch size
Q = 64            # quadrant offset for second lane (dir-b / sweep s2)
KH = 114          # K rows for rec matmuls / h span (0:50 real, 64:114 real)
KH1 = 115         # K rows for L1 proj (incl. ones row at 114)

# PyTorch LSTM gate order in weight rows: i, f, g, o — bank order matches.
GATE_SLICES = [slice(0, 50), slice(50, 100), slice(100, 150), slice(150, 200)]
# g-bank pre-activations are scaled x2 so sigmoid(2g) = (tanh(g)+1)/2.
GATE_SCALE = [1.0, 1.0, 2.0, 1.0]


def _pack_weights(inp):
    f32 = np.float32
    # ---- L0 projection (+bias via ones row), [208, 4*128] ----
    w0 = np.zeros((DK, 512), f32)
    wf = np.concatenate([inp["wih0f"], inp["b0f"][:, None]], axis=1)  # [200,208]
    wb = np.concatenate([inp["wih0b"], inp["b0b"][:, None]], axis=1)
    for c, gs in enumerate(GATE_SLICES):
        w0[:, c * 128 + 0:c * 128 + 50] = GATE_SCALE[c] * wf[gs].T
        w0[:, c * 128 + Q:c * 128 + Q + 50] = GATE_SCALE[c] * wb[gs].T
    # ---- L0 recurrent, block-diag [114, 4*128] ----
    r0 = np.zeros((KH, 512), f32)
    for c, gs in enumerate(GATE_SLICES):
        r0[0:50, c * 128 + 0:c * 128 + 50] = GATE_SCALE[c] * inp["whh0f"][gs].T
        r0[Q:Q + 50, c * 128 + Q:c * 128 + Q + 50] = GATE_SCALE[c] * inp["whh0b"][gs].T
    # ---- L1 fwd projection [115, 4*64] (rows: h1f 0:50, h1b 64:114, bias 114)
    def l1_proj(wih, b):
        w = np.zeros((KH1, 256), f32)
        for c, gs in enumerate(GATE_SLICES):
            w[0:50, c * 64:c * 64 + 50] = GATE_SCALE[c] * wih[gs, 0:50].T
            w[Q:Q + 50, c * 64:c * 64 + 50] = GATE_SCALE[c] * wih[gs, 50:100].T
            w[KH, c * 64:c * 64 + 50] = GATE_SCALE[c] * b[gs]
        return w
    w1 = l1_proj(inp["wih1f"], inp["b1f"])
    w1b = l1_proj(inp["wih1b"], inp["b1b"])
    # ---- L1 recurrent, block-diag per sweep pair [114, 4*128] ----
    r1 = np.zeros((KH, 512), f32)
    for c, gs in enumerate(GATE_SLICES):
        r1[0:50, c * 128 + 0:c * 128 + 50] = GATE_SCALE[c] * inp["whh1f"][gs].T
        r1[Q:Q + 50, c * 128 + Q:c * 128 + Q + 50] = GATE_SCALE[c] * inp["whh1f"][gs].T
    # ---- FC ----
    # FC weights pre-staged at both quadrants host-side: a [50,2] DMA has
    # 4-byte lines and cost ~13us on hardware; one [128,2] DMA is cheap.
    wffq = np.zeros((128, 2), f32)
    wfbq = np.zeros((128, 2), f32)
    wffq[0:50] = wffq[Q:Q + 50] = inp["fc_w"][:, 0:50].T
    wfbq[0:50] = wfbq[Q:Q + 50] = inp["fc_w"][:, 50:100].T
    f16 = np.float16
    return {
        "w0hi": w0[0:128].astype(f16), "w0lo": w0[128:DK].astype(f16),
        "r0": r0.astype(f16), "w1": w1.astype(f16), "w1b": w1b.astype(f16),
        "r1": r1.astype(f16), "wffq": wffq.astype(f16), "wfbq": wfbq.astype(f16),
        # h1 scratch-row init blocks, DMA'd instead of memset (engines are
        # slow at big memsets; DMA rings are idle at startup).
        "h1z": np.zeros((32, T, BS), f16),
        "h1o": np.ones((32, T, BS), f16),
    }


def _pack_x(x):
    # x [B, T, 207] f32 -> [T, 208, B] fp16 with ones row at d=207
    xt = np.empty((T, DK, B), dtype=np.float16)
    xt[:, 0:DIN, :] = x.transpose(1, 2, 0).astype(np.float16)
    xt[:, DIN, :] = np.float16(1.0)
    return xt


def _elide_redundant_ldweights(nc):
    """Remove InstLdweights whose weights are already resident in the same
    PE col-range (bass emits one LDWEIGHTS per matmul unconditionally).

    Back-to-back same-weight matmuls then pipeline fill-behind-drain
    instead of paying the full drain + reload between them. Waits/updates
    on a removed LDW migrate to the next PE instruction (its matmul)."""
    removed = skipped = 0
    for blk in nc.m.functions[0].blocks:
        insts = blk.instructions
        state = {}  # col_start -> (width, signature)
        to_remove = []
        for idx, inst in enumerate(insts):
            if not isinstance(inst, mybir.InstLdweights):
                continue
            tp = getattr(inst, "tile_position", None) or (0, 0)
            tsz = getattr(inst, "tile_size", None) or (128, 128)
            col0, width = tp[1], tsz[1]
            sig = (str(inst.ins[0]), str(getattr(inst, "perf_mode", None)),
                   str(getattr(inst, "is_transpose", None)), width)
            prev = state.get(col0)
            if prev == sig:
                # find next PE instruction (the matmul this LDW feeds)
                si = inst.sync_info
                nxt = None
                for j in range(idx + 1, len(insts)):
                    if insts[j].engine == mybir.EngineType.PE:
                        nxt = insts[j]
                        break
                if nxt is None:
                    skipped += 1
                    continue
                nsi = nxt.sync_info
                lw = list(si.on_wait) if si else []
                lu = list(si.on_update) if si else []
                nw = list(nsi.on_wait) if nsi else []
                if lw and nw:
                    skipped += 1  # can't merge two waits; keep the LDW
                    continue
                if lw or lu:
                    nu = list(nsi.on_update) if nsi else []
                    nxt.sync_info = mybir.SyncInfo(
                        on_wait=nw + lw, on_update=nu + lu)
                to_remove.append(idx)
            else:
                # invalidate anything overlapping this col range, then record
                for c0 in list(state):
                    w0 = state[c0][3] if isinstance(state[c0], tuple) else 128
                    if c0 < col0 + width and col0 < c0 + w0:
                        del state[c0]
                state[col0] = sig
        for idx in reversed(to_remove):
            del insts[idx]
        removed += len(to_remove)
    print(f"ldweights elision: removed {removed}, kept-for-waits {skipped}")


def _build_nc():
    nc = bacc.Bacc("TRN2", target_bir_lowering=False, debug=False)
    ap = {}
    ap["xT"] = nc.dram_tensor("xT", [T, DK, BC], F16, kind="ExternalInput").ap()
    for name, shp in [("w0hi", [128, 512]), ("w0lo", [DK - 128, 512]),
                      ("r0", [KH, 512]), ("w1", [KH1, 256]), ("w1b", [KH1, 256]),
                      ("r1", [KH, 512]), ("wffq", [128, 2]), ("wfbq", [128, 2]),
                      ("h1z", [32, T, BS]), ("h1o", [32, T, BS])]:
        ap[name] = nc.dram_tensor(name, shp, F16, kind="ExternalInput").ap()
    out_ap = nc.dram_tensor("out", [2, BC], F32, kind="ExternalOutput").ap()

    with tile.TileContext(nc) as tc:
        with (
            tc.tile_pool(name="wts", bufs=1) as wp,
            tc.tile_pool(name="xin", bufs=2) as xp,
            tc.tile_pool(name="h1p", bufs=1) as h1p,
            tc.tile_pool(name="st", bufs=1) as sp,
            tc.tile_pool(name="gt", bufs=1) as gp,
            tc.tile_pool(name="outp", bufs=2) as op_,
        ):
            # ---- load weights to SBUF ----
            # Phase-A weights on sync (needed immediately, ahead of x loads);
            # layer-1 weights on gpsimd; FC weights on the idle scalar queue
            # (a slow 4-byte-line DMA there blocked startup for ~19us).
            w = {}
            for name, shp, eng in [
                ("w0hi", [128, 512], nc.sync), ("w0lo", [DK - 128, 512], nc.sync),
                ("r0", [KH, 512], nc.sync), ("w1", [KH1, 256], nc.gpsimd),
                ("w1b", [KH1, 256], nc.gpsimd), ("r1", [KH, 512], nc.gpsimd),
                ("wffq", [128, 2], nc.scalar), ("wfbq", [128, 2], nc.scalar),
            ]:
                t = wp.tile(shp, F16, tag=name, name=name)
                eng.dma_start(t[:], ap[name][:])
                w[name] = t

            # ---- h1 history per sweep [115, T, 512]; row 114 = ones ----
            h1 = []
            for s in range(NSW):
                t = h1p.tile([128, T, BS], F16, tag=f"h1_{s}", name=f"h1_{s}")
                # ones row lives at 114; init needs a 32-aligned partition
                # start, so fill 96:128 — Phase A's h1 DMA re-writes 96:114
                # before layer 1 reads, and 115:128 is outside the K span.
                # DMA'd from host constants: engine memsets of this size cost
                # ~13us each and serialize startup; DMA rings are idle here.
                eng = nc.scalar if s < 2 else nc.gpsimd
                eng.dma_start(t[96:128, :, :], ap["h1o"][:])
                # rows 50:64 are never DMA'd; zero them so the 0-weight
                # K rows in layer-1 matmuls read 0 (not NaN). 32:50 gets
                # overwritten by the fwd-lane h1 DMA each step.
                eng.dma_start(t[32:64, :, :], ap["h1z"][:])
                h1.append(t)

            # ---- states (all fp16 for 2x DVE modes) ----
            hS = [sp.tile([128, BS], F16, tag=f"hs{s}", name=f"hs{s}") for s in range(NSW)]
            # c / tanh(c) live in per-pair tiles so one ACTIVATE covers both
            # in-flight sweeps of a pair.
            cP = [sp.tile([128, 2, BS], F16, tag=f"cp{r}", name=f"cp{r}") for r in range(2)]
            tP = [sp.tile([128, 2, BS], F16, tag=f"tp{r}", name=f"tp{r}") for r in range(2)]
            hB = [sp.tile([128, BS], F16, tag=f"hb{p}", name=f"hb{p}") for p in range(2)]
            hC = [sp.tile([128, BS], F16, tag=f"hc{p}", name=f"hc{p}") for p in range(2)]
            cB = [sp.tile([128, BS], F16, tag=f"cb{p}", name=f"cb{p}") for p in range(2)]
            tB = [sp.tile([128, BS], F16, tag=f"tb{p}", name=f"tb{p}") for p in range(2)]
            cC = [sp.tile([128, BS], F16, tag=f"cc{p}", name=f"cc{p}") for p in range(2)]
            tC = [sp.tile([128, BS], F16, tag=f"tc{p}", name=f"tc{p}") for p in range(2)]

            def cell_cupd(G, c_t, k):
                """c'-update on a HALF-SCALED cell state c' = c/2:
                t = (s-0.5)*i == i*tanh(g)/2, c' = f*c' + t. The x2 rides
                tanh's free scale later. g-weights are pre-scaled x2 so the
                g-bank sigmoid s gives tanh(g) = 2s-1. Plain tensor_tensor
                runs at 2x on fp16; scalar_tensor_tensor only has a 1x uop,
                so keep just one of those."""
                i_, f_, g_ = (G[0:KH, j, :] for j in range(3))
                c_ = c_t[0:KH, :]
                if k == 0:
                    nc.vector.scalar_tensor_tensor(c_, g_, 0.5, i_,
                                                   OP.subtract, OP.mult)
                else:
                    nc.vector.tensor_mul(c_, f_, c_)
                    nc.vector.scalar_tensor_tensor(g_, g_, 0.5, i_,
                                                   OP.subtract, OP.mult)
                    nc.vector.tensor_add(c_, c_, g_)

            def cell(Pif, Pgo, G, c_t, k):
                """Split-PSUM cell front: two 2-bank sigmoids (i,f) / (2g,o)
                so f*c can start as soon as the first one lands. tanh + h
                are emitted by the caller (batched per pair in phase A)."""
                nc.scalar.activation(G[0:KH, 0:2, :], Pif[0:KH, :, :], AF.Sigmoid)
                nc.scalar.activation(G[0:KH, 2:4, :], Pgo[0:KH, :, :], AF.Sigmoid)
                cell_cupd(G, c_t, k)

            def cell42(P4, G, c_t, th_t, h_out, k):
                """Phase B/C cell on a single 4-bank tile, sigmoid split in
                two so the c-chain starts after the first half (the layer-1
                scan is latency-bound, not ACT-throughput-bound)."""
                nc.scalar.activation(G[0:KH, 0:2, :], P4[0:KH, 0:2, :], AF.Sigmoid)
                nc.scalar.activation(G[0:KH, 2:4, :], P4[0:KH, 2:4, :], AF.Sigmoid)
                cell_cupd(G, c_t, k)
                nc.scalar.activation(th_t[0:KH, :], c_t[0:KH, :], AF.Tanh,
                                     scale=2.0)
                nc.vector.tensor_mul(h_out[0:KH, :], G[0:KH, 3, :],
                                     th_t[0:KH, :])

            with (
                tc.tile_pool(name="psA", bufs=2, space="PSUM") as ppA,
                tc.tile_pool(name="psB", bufs=2, space="PSUM") as ppB,
            ):
                def gate_tiles():
                    """Two half-tiles per sweep: banks (i,f) and (g,o).
                    Each recycles as soon as its own sigmoid drains it."""
                    pif = ppA.tile([128, 2, BS], F32, tag="pif", name="pif")
                    pgo = ppB.tile([128, 2, BS], F32, tag="pgo", name="pgo")
                    return pif, pgo

                def bank(pair, c, rows):
                    pif, pgo = pair
                    return (pif if c < 2 else pgo)[rows, c % 2, :]

                # =================== Phase A: layer 0, lockstep ===========
                for k in range(T):
                    # one x DMA per (step, kind) covering all 4 sweeps;
                    # fwd loads on sync queue, bwd loads on gpsimd queue.
                    xfh = xp.tile([128, BC], F16, tag="xfh", name="xfh")
                    nc.sync.dma_start(xfh[:], ap["xT"][k, 0:128, :])
                    xfl = xp.tile([DK - 128, BC], F16, tag="xfl", name="xfl")
                    nc.sync.dma_start(xfl[:], ap["xT"][k, 128:DK, :])
                    xbh = xp.tile([128, BC], F16, tag="xbh", name="xbh")
                    nc.gpsimd.dma_start(xbh[:], ap["xT"][T - 1 - k, 0:128, :])
                    xbl = xp.tile([DK - 128, BC], F16, tag="xbl", name="xbl")
                    nc.gpsimd.dma_start(xbl[:], ap["xT"][T - 1 - k, 128:DK, :])

                    for pr in range(2):
                        sa, sb = 2 * pr, 2 * pr + 1
                        Ps = [gate_tiles() for _ in range(2)]
                        # Rotated interleave: adjacent MMs hit disjoint
                        # col-groups (concurrent); same-weight MMs for the
                        # two in-flight sweeps are back to back.
                        for wt_, xfx, xbx, st in ((w["w0hi"], xfh, xbh, True),
                                                  (w["w0lo"], xfl, xbl, False)):
                            for c in range(4):
                                cb = (c + 2) % 4
                                spA = k == 0 and not st and c >= 2
                                spB = k == 0 and not st and cb < 2
                                for sj, s in enumerate((sa, sb)):
                                    sl = bass.ts(s, BS)
                                    nc.tensor.matmul(
                                        bank(Ps[sj], c, slice(0, Q)),
                                        wt_[:, bass.ds(c * 128, Q)], xfx[:, sl],
                                        start=st, stop=spA)
                                    nc.tensor.matmul(
                                        bank(Ps[sj], cb, slice(Q, 128)),
                                        wt_[:, bass.ds(cb * 128 + Q, Q)],
                                        xbx[:, sl],
                                        start=st, stop=spB)
                        if k > 0:
                            for c in range(4):
                                for sj, s in enumerate((sa, sb)):
                                    nc.tensor.matmul(
                                        bank(Ps[sj], c, slice(0, 128)),
                                        w["r0"][:, bass.ts(c, 128)],
                                        hS[s][0:KH, :],
                                        start=False, stop=True)

                        for sj, s in enumerate((sa, sb)):
                            G = gp.tile([128, 4, BS], F16, tag=f"g{s}", name=f"g{s}")
                            cell(Ps[sj][0], Ps[sj][1], G, cP[pr][:, sj, :], k)
                            # x2 undoes the half-scaled c' state; per-sweep
                            # tanh keeps the two chains decoupled.
                            nc.scalar.activation(tP[pr][0:KH, sj, :],
                                                 cP[pr][0:KH, sj, :], AF.Tanh,
                                                 scale=2.0)
                            nc.vector.tensor_mul(hS[s][0:KH, :],
                                                 G[0:KH, 3, :],
                                                 tP[pr][0:KH, sj, :])
                            # h1 history is time-aligned: bwd lane at step k
                            # holds h_b for time T-1-k.
                            nc.sync.dma_start(h1[s][0:50, k, :], hS[s][0:50, :])
                            nc.gpsimd.dma_start(h1[s][Q:Q + 50, T - 1 - k, :],
                                                hS[s][Q:Q + 50, :])

            # =================== Phase B: layer 1 forward =============
            with tc.tile_pool(name="ps4", bufs=2, space="PSUM") as pp4:
                for k in range(T):
                    Ps = [pp4.tile([128, 4, BS], F32, tag="p4", name="p4")
                          for _ in range(2)]
                    for c in range(4):
                        cb = (c + 2) % 4
                        for p in range(2):
                            s1, s2 = 2 * p, 2 * p + 1
                            nc.tensor.matmul(
                                Ps[p][0:Q, c, :],
                                w["w1"][:, bass.ts(c, 64)],
                                h1[s1][0:KH1, k, :], start=True,
                                stop=(k == 0 and c >= 2))
                            nc.tensor.matmul(
                                Ps[p][Q:128, cb, :],
                                w["w1"][:, bass.ts(cb, 64)],
                                h1[s2][0:KH1, k, :], start=True,
                                stop=(k == 0 and cb < 2))
                    if k > 0:
                        for c in range(4):
                            for p in range(2):
                                nc.tensor.matmul(
                                    Ps[p][:, c, :],
                                    w["r1"][:, bass.ts(c, 128)], hB[p][0:KH, :],
                                    start=False, stop=True)
                    for p in range(2):
                        G = gp.tile([128, 4, BS], F16, tag=f"g{p}", name=f"g{p}")
                        cell42(Ps[p], G, cB[p], tB[p], hB[p], k)

                # =================== Phase C: layer 1 backward (t=29) =====
                for p in range(2):
                    s1, s2 = 2 * p, 2 * p + 1
                    P = pp4.tile([128, 4, BS], F32, tag="p4", name="p4")
                    for c in range(4):
                        cb = (c + 2) % 4
                        nc.tensor.matmul(
                            P[0:Q, c, :], w["w1b"][:, bass.ts(c, 64)],
                            h1[s1][0:KH1, T - 1, :], start=True, stop=c >= 2)
                        nc.tensor.matmul(
                            P[Q:128, cb, :], w["w1b"][:, bass.ts(cb, 64)],
                            h1[s2][0:KH1, T - 1, :], start=True, stop=cb < 2)
                    G = gp.tile([128, 4, BS], F16, tag=f"g{p + 2}", name=f"g{p + 2}")
                    cell42(P, G, cC[p], tC[p], hC[p], 0)

            # =================== FC ====================================
            with tc.tile_pool(name="fcp", bufs=4, space="PSUM") as fcp:
                for s in range(NSW):
                    p, qo = s // 2, (s % 2) * Q
                    F = fcp.tile([2, BS], F32, tag="fc", name="fc")
                    nc.tensor.matmul(F[:], w["wffq"][qo:qo + 50, :],
                                     hB[p][qo:qo + 50, :], start=True, stop=False)
                    nc.tensor.matmul(F[:], w["wfbq"][qo:qo + 50, :],
                                     hC[p][qo:qo + 50, :], start=False, stop=True)
                    ot = op_.tile([2, BS], F32, tag="o", name="ot")
                    nc.vector.tensor_copy(ot[:], F[:])
                    nc.sync.dma_start(out_ap[:, bass.ts(s, BS)], ot[:])

    nc.compile()
    _elide_redundant_ldweights(nc)
    return nc


_NC_CACHE = None


def kernel(**inputs) -> np.ndarray:
    global _NC_CACHE
    if _NC_CACHE is None:
        _NC_CACHE = _build_nc()
    nc = _NC_CACHE
    wts = _pack_weights(inputs)
    xt = _pack_x(np.asarray(inputs["x"], dtype=np.float32))
    in_maps = []
    for c in range(NCORES):
        m = dict(wts)
        m["xT"] = np.ascontiguousarray(xt[:, :, c * BC:(c + 1) * BC])
        in_maps.append(m)
    res = run_bass_kernel_spmd(nc, in_maps, list(range(NCORES)))
    outs = [res.results[c]["out"] for c in range(NCORES)]  # [2, BC] each
    full = np.concatenate(outs, axis=1).T  # [B, 2]
    return (full + inputs["fc_b"][None, :]).astype(np.float32)
